# revision 10
# baseline (speedup 1.0000x reference)
"""GCN-Attention kernel for Trainium2, data-parallel over 8 NeuronCores.

Reference computation (per image b of 64, category c of 100):
  full = concat(image_features, bbox)                    [N, 2052]
  x[b,c,:] = sum_{boxes n in bucket(b,c), slot<3} lin_w[slot]*full[n] + lin_b
  support  = x @ gc_w                                    [B, 100, 2048]
  gcn      = leaky_relu((X + adj) @ support + gc_b)
  out[b]   = global_features[b] @ gcn[b]                 [B, 2048]

Host prep (pure input reorganization, <0.3% of total FLOPs): the occurrence-
slot scatter is resolved into the weighted sum x on the host; the lin_b bias
becomes a constant-ones contraction row paired with lin_b*colsum(gc_w).

Device mapping (per core, 8 images = 800 (image,category) rows), bf16
matmuls with fp32 PSUM accumulate:
  phase 2: the 800 rows are packed densely into 7 partition tiles of <=128,
           support tile [<=128, 2048] = x^T_k (stationary) x gc_w_k (moving)
           accumulated over 16 K=128 chunks + one K=5 chunk (4 bbox features
           + the lin_b ones-row), 4 interleaved PSUM chains (one per 512-col
           bank) so same-bank accumulating matmuls stay 4 issues apart.
  phase 3: per image, adjT (stationary) x support rows (moving); images
           whose 100 rows straddle a packed-tile boundary use 2 accumulating
           matmuls; Lrelu on the scalar engine; emitted one tile late so the
           casts are long done.
  phase 4: attention row matmul, DVE copy into a [1,2048] staging row,
           one 8KB output DMA per image; emitted two tiles late so the
           Lrelu latency hides under phase-2 matmuls.

DMA layout: all bulk inputs are shipped in partition-major pair-chunk form
(gc_w as 8 x [128, 8KB-contiguous-per-partition], x^T as 8 x [128, 3.2KB])
so each descriptor moves a large contiguous line; pairs round-robin over 4
engine queues, so compute starts ~4us in and never starves.
"""
import os
import time

import ml_dtypes
import numpy as np

import concourse.bacc as bacc
import concourse.mybir as mybir
import concourse.tile as tile
from concourse import bass_utils

B = 64
C = 100
LOOP = 3
FEAT = 2052
OUT = 2048
NCORES = 8
BPC = B // NCORES  # images per core
ROWS = BPC * C     # packed (image,category) rows per core
NMT = (ROWS + 127) // 128  # 7 packed row tiles
NKP = 8            # gc_w / x^T pair chunks (2 x 128 rows each)
NCH = 4            # 512-col output chunks

f32 = mybir.dt.float32
bf16 = mybir.dt.bfloat16
np_bf16 = ml_dtypes.bfloat16

_programs: dict = {}
last_results = None  # BassKernelResults of the most recent run (for harnesses)


def _occ_slots(key):
    """Occurrence index among equal-valued keys, stable order (matches jax ref)."""
    n = key.shape[0]
    order = np.argsort(key, kind="stable")
    sk = key[order]
    idx = np.arange(n)
    is_new = np.concatenate([[True], sk[1:] != sk[:-1]]) if n else np.zeros(0, bool)
    run_start = np.maximum.accumulate(np.where(is_new, idx, 0))
    pos = idx - run_start
    slots = np.zeros(n, np.int64)
    slots[order] = pos
    return slots


def _mt_width(m):
    return min(128, ROWS - m * 128)


def _img_segments():
    """Per image: (tile, p_lo, p_hi, c0) contraction segments.

    Matmul operands must sit at base partition 0, so each segment's
    stationary adjT block is shipped zero-padded from partition 0 up to
    p_lo; the moving ssb read covers [0, p_hi) and foreign images' rows
    multiply the zero rows.
    """
    segs = []
    for b in range(BPC):
        g0 = b * C
        u0, p0 = g0 // 128, g0 % 128
        k1 = min(C, 128 - p0)
        s = [(u0, p0, p0 + k1, 0)]
        if k1 < C:
            s.append((u0 + 1, 0, C - k1, k1))
        segs.append(s)
    return segs


ADJ_SEGS = _img_segments()
ADJ_NSEG = sum(len(s) for s in ADJ_SEGS)
# flat column-block index of each image's first segment in the packed adjT
ADJ_OFF = np.cumsum([0] + [len(s) for s in ADJ_SEGS]).tolist()


def _build_packed(has_gcb: bool):
    nc = bacc.Bacc("TRN2", target_bir_lowering=False, debug=False,
                   num_devices=NCORES)

    # pair-chunk inputs: partition line j of gcwp holds gc_w rows
    # [256j+p ; 256j+128+p] (8KB contiguous), ditto xtp for x^T
    gcwp_d = nc.dram_tensor("gcwp", [NKP, 128, 2 * OUT], bf16, kind="ExternalInput").ap()
    xtp_d = nc.dram_tensor("xtp", [NKP, 128, 2 * ROWS], bf16, kind="ExternalInput").ap()
    gcw5_d = nc.dram_tensor("gcw5", [5, OUT], bf16, kind="ExternalInput").ap()
    xt5_d = nc.dram_tensor("xt5", [5, ROWS], bf16, kind="ExternalInput").ap()
    adjT_d = nc.dram_tensor("adjT", [128, ADJ_NSEG * C], bf16,
                            kind="ExternalInput").ap()
    gT_d = nc.dram_tensor("gT", [C, BPC], bf16, kind="ExternalInput").ap()
    if has_gcb:
        gcbr_d = nc.dram_tensor("gcbr", [1, OUT], bf16, kind="ExternalInput").ap()
        ones_d = nc.dram_tensor("ones", [1, C], bf16, kind="ExternalInput").ap()
    out_d = nc.dram_tensor("out", [BPC, OUT], f32, kind="ExternalOutput").ap()

    # images whose last packed row lands in tile m
    done_at = [[] for _ in range(NMT)]
    for b in range(BPC):
        done_at[(b * C + C - 1) // 128].append(b)

    dmaq = None

    with tile.TileContext(nc) as tc:
        with tc.tile_pool(name="const", bufs=1) as cpool, \
             tc.tile_pool(name="sb", bufs=1) as pool, \
             tc.tile_pool(name="ps", bufs=1, space="PSUM") as psp:

            dmaq = [nc.sync, nc.scalar, nc.gpsimd]

            # interleave [gcw pair, xt pair] per queue so the low-k chunks
            # phase 2 consumes first are also the first to land
            gcwp_sb, xtp_sb = [None] * NKP, [None] * NKP
            for j in range(NKP):
                q = dmaq[j % 3]
                gt = cpool.tile([128, 2 * OUT], bf16, tag=f"gcwp{j}",
                                name=f"gcwp_sb{j}")
                q.dma_start(gt[:], gcwp_d[j])
                gcwp_sb[j] = gt
                xt = cpool.tile([128, 2 * ROWS], bf16, tag=f"xtp{j}",
                                name=f"xtp_sb{j}")
                q.dma_start(xt[:], xtp_d[j])
                xtp_sb[j] = xt
            gcw5_sb = cpool.tile([5, OUT], bf16, tag="gcw5")
            nc.sync.dma_start(gcw5_sb[:], gcw5_d[:])
            xt5_sb = cpool.tile([5, ROWS], bf16, tag="xt5")
            nc.scalar.dma_start(xt5_sb[:], xt5_d[:])
            adjT_sb = cpool.tile([128, ADJ_NSEG * C], bf16, tag="adjT")
            nc.gpsimd.dma_start(adjT_sb[:], adjT_d[:])
            gT_sb = cpool.tile([C, BPC], bf16, tag="gT")
            nc.sync.dma_start(gT_sb[:], gT_d[:])
            if has_gcb:
                gcbr_sb = cpool.tile([1, OUT], bf16, tag="gcbr")
                nc.sync.dma_start(gcbr_sb[:], gcbr_d[:])
                ones_sb = cpool.tile([1, C], bf16, tag="ones")
                nc.scalar.dma_start(ones_sb[:], ones_d[:])

            def stat_slice(k, m, mw):
                # stationary x^T chunk k for row tile m
                if k == 16:
                    return xt5_sb[0:5, m * 128:m * 128 + mw]
                t = xtp_sb[k // 2]
                off = (k % 2) * ROWS
                return t[0:128, off + m * 128:off + m * 128 + mw]

            def mov_slice(k, j):
                # moving gc_w chunk k, output chunk j
                if k == 16:
                    return gcw5_sb[0:5, j * 512:(j + 1) * 512]
                t = gcwp_sb[k // 2]
                off = (k % 2) * OUT
                return t[0:128, off + j * 512:off + j * 512 + 512]

            def phase2(m):
                mw = _mt_width(m)
                sps = [psp.tile([128, 512], f32, tag="sp", bufs=5,
                                name=f"sp_{m}_{j}") for j in range(NCH)]
                for k in range(17):
                    kw = 5 if k == 16 else 128
                    for j in range(NCH):
                        nc.tensor.matmul(
                            sps[j][0:mw, 0:512],
                            stat_slice(k, m, mw),
                            mov_slice(k, j),
                            start=(k == 0), stop=(k == 16),
                        )
                ssb = pool.tile([128, OUT], bf16, tag="ssb", bufs=NMT,
                                name=f"ssb_{m}")
                for j in range(NCH):
                    nc.vector.tensor_copy(ssb[0:mw, j * 512:(j + 1) * 512],
                                          sps[j][0:mw, 0:512])
                return ssb

            def phase3(b, ssbs):
                gsb = pool.tile([C, OUT], bf16, tag="gsb", bufs=3,
                                name=f"gsb_{b}")
                for j in range(NCH):
                    gp = psp.tile([128, 512], f32, tag="gp", bufs=2,
                                  name=f"gp_{b}_{j}")
                    ns = len(ADJ_SEGS[b]) + (1 if has_gcb else 0)
                    for i, (u, p_lo, p_hi, c0) in enumerate(ADJ_SEGS[b]):
                        s = ADJ_OFF[b] + i
                        nc.tensor.matmul(
                            gp[0:C, 0:512],
                            adjT_sb[0:p_hi, s * C:(s + 1) * C],
                            ssbs[u][0:p_hi, j * 512:(j + 1) * 512],
                            start=(i == 0), stop=(i == ns - 1),
                        )
                    if has_gcb:
                        nc.tensor.matmul(
                            gp[0:C, 0:512], ones_sb[0:1, 0:C],
                            gcbr_sb[0:1, j * 512:(j + 1) * 512],
                            start=False, stop=True,
                        )
                    nc.scalar.activation(
                        gsb[0:C, j * 512:(j + 1) * 512], gp[0:C, 0:512],
                        mybir.ActivationFunctionType.Lrelu, alpha=0.01,
                    )
                return gsb

            def phase4(b, gsb):
                ostage = pool.tile([1, OUT], f32, tag="ostage", bufs=2,
                                   name=f"ost_{b}")
                for j in range(NCH):
                    op = psp.tile([1, 512], f32, tag="op", bufs=1,
                                  name=f"op_{b}_{j}")
                    nc.tensor.matmul(op[0:1, 0:512], gT_sb[0:C, b:b + 1],
                                     gsb[0:C, j * 512:(j + 1) * 512],
                                     start=True, stop=True)
                    nc.vector.tensor_copy(ostage[0:1, j * 512:(j + 1) * 512],
                                          op[0:1, 0:512])
                nc.gpsimd.dma_start(out_d[b:b + 1, :], ostage[0:1, :])

            # software pipeline: phase 3 one row-tile late, phase 4 two late
            ssbs = [None] * NMT
            gsbs = [None] * BPC
            for m in range(NMT):
                ssbs[m] = phase2(m)
                if m >= 1:
                    for b in done_at[m - 1]:
                        gsbs[b] = phase3(b, ssbs)
                if m >= 2:
                    for b in done_at[m - 2]:
                        phase4(b, gsbs[b])
            for b in done_at[NMT - 1]:
                gsbs[b] = phase3(b, ssbs)
            for b in done_at[NMT - 2] + done_at[NMT - 1]:
                phase4(b, gsbs[b])

    nc.compile()
    return nc


def _get_program(has_gcb: bool = False):
    key = ("packed", has_gcb)
    if key not in _programs:
        _programs[key] = _build_packed(has_gcb)
    return _programs[key]


def kernel(**inputs) -> np.ndarray:
    global last_results

    imf = np.asarray(inputs["image_features"], np.float32)
    bbox = np.asarray(inputs["bbox_list"], np.float32)
    gf = np.asarray(inputs["global_features"], np.float32)
    adj = np.asarray(inputs["adj"], np.float32)
    X = np.asarray(inputs["X"], np.float32)
    lin_w = np.asarray(inputs["lin_w"], np.float32)
    lin_b = np.float32(np.asarray(inputs["lin_b"]))
    gc_w = np.ascontiguousarray(np.asarray(inputs["gc_w"], np.float32))
    gc_b = np.asarray(inputs["gc_b"], np.float32)
    label = np.asarray(inputs["label_list"]).astype(np.int64)
    batch = np.asarray(inputs["batch"]).astype(np.int64)

    full = np.concatenate([imf, bbox], axis=1)

    # scatter bookkeeping, matching jax semantics: slots by stable order of
    # key=batch*C+(label-1); negative cats wrap, slot>=LOOP / far-oob dropped
    cat = label - 1
    key = batch * C + cat
    slots = _occ_slots(key)
    valid = (slots < LOOP) & (cat >= -C) & (cat < C)
    wvals = np.where(valid, lin_w[np.clip(slots, 0, LOOP - 1)], 0.0).astype(np.float32)
    cidx = np.mod(cat, C).astype(np.int64)

    # host scatter-sum (0.04% of total FLOPs): S[b,c,:] = sum of
    # lin_w[slot]*full over the <=LOOP boxes of bucket (b,c); slots are
    # unique per bucket so per-slot fancy-index adds have no collisions
    S = np.zeros((B, C, FEAT), np.float32)
    bok = valid & (batch >= -B) & (batch < B)
    bmod = np.mod(batch, B)
    for s in range(LOOP):
        sel = bok & (slots == s)
        if np.any(sel):
            S[bmod[sel], cidx[sel]] += wvals[sel, None] * full[sel]

    newadj = X[None, :, :] + adj                               # [B, C, C]
    has_gcb = bool(np.any(gc_b))

    # gc_w pair chunks: line p of pair j = rows [256j+p ; 256j+128+p]
    gcwp = np.ascontiguousarray(
        gc_w[0:2048].reshape(NKP, 2, 128, OUT).swapaxes(1, 2).reshape(
            NKP, 128, 2 * OUT)).astype(np_bf16)
    gcw5 = np.concatenate(
        [gc_w[2048:FEAT], (lin_b * gc_w.sum(axis=0))[None, :]]).astype(np_bf16)

    in_maps = []
    for core in range(NCORES):
        imgs = slice(core * BPC, (core + 1) * BPC)
        Xc = S[imgs].reshape(ROWS, FEAT)
        XT = np.ascontiguousarray(Xc[:, 0:2048].T)             # [2048, 800]
        xtp = np.ascontiguousarray(
            XT.reshape(NKP, 2, 128, ROWS).swapaxes(1, 2).reshape(
                NKP, 128, 2 * ROWS)).astype(np_bf16)
        xt5 = np.concatenate(
            [Xc[:, 2048:FEAT].T, np.ones((1, ROWS), np.float32)]).astype(np_bf16)
        # packed adjT blocks: segment s of image b holds A'_b[c', c]^T rows
        # for packed partitions [p_lo, p_hi), zero-padded down to partition 0
        adjT_pack = np.zeros((128, ADJ_NSEG * C), np.float32)
        for b in range(BPC):
            Ab = newadj[core * BPC + b]                       # [c', c]
            for i, (u, p_lo, p_hi, c0) in enumerate(ADJ_SEGS[b]):
                s = ADJ_OFF[b] + i
                nrows = p_hi - p_lo
                adjT_pack[p_lo:p_hi, s * C:(s + 1) * C] = \
                    Ab[:, c0:c0 + nrows].T
        im = dict(
            gcwp=gcwp, gcw5=gcw5, xtp=xtp, xt5=xt5,
            adjT=adjT_pack.astype(np_bf16),
            gT=np.ascontiguousarray(gf[imgs].T).astype(np_bf16),
        )
        if has_gcb:
            im["gcbr"] = gc_b[None, :].astype(np_bf16)
            im["ones"] = np.ones((1, C), np_bf16)
        in_maps.append(im)

    nc = _get_program(has_gcb)
    res = None
    for attempt in range(4):
        try:
            res = bass_utils.run_bass_kernel_spmd(
                nc, in_maps, core_ids=list(range(NCORES)))
            break
        except Exception:
            if attempt == 3:
                raise
            time.sleep(3 * (attempt + 1))  # transient NRT exec-unit errors
    last_results = res
    return np.concatenate([res.results[i]["out"] for i in range(NCORES)], axis=0)
